# revision 10
# baseline (speedup 1.0000x reference)
"""Trainium2 Bass kernel for nn_CrossAttention_31078383354530.

Reference computation (b=2, n=m=2048, qd=1024, cd=768, heads=8, dh=128):
    q = x @ Wq; k = ctx @ Wk; v = ctx @ Wv  (split into 8 heads of 128)
    sim = (q @ k^T) * dh**-0.5 over the FLATTENED (b*n)=4096 token axis
    attn = softmax((sim - mean)*1.5 + mean) == softmax(1.5*scale*(q@k^T))
        exactly (the mean-centering is a per-row constant shift)
    out = attn @ v -> merge heads -> y = out @ Wout + bout

Sharding (8 cores): context-token-sharded K/V projection + AllGather of the
bf16 K/V (all heads), then each core runs all 8 heads' attention for its own
512-query-token slice and its own final projection -> the output is a
disjoint row-slice per core (no reduction needed on host).

Schedule notes (v2):
  * A zero-byte dummy AllGather is issued at t=0 so the first-collective
    entry barrier overlaps the projection phase.
  * K/V projection + AllGather are pipelined per head-PAIR (4 collectives),
    each launched as soon as its 2 heads' K/V slices are in DRAM.
  * The softmax row-sum no longer streams the attention tiles through the
    PE a second time: the DVE accumulates the exp tiles into two bf16
    accumulators (2x perf mode) and a single ones-stationary matmul per
    head reduces those across partitions.  This removes 1/3 of the
    attention-phase PE work; the kernel is then paced by the Scalar
    engine's exp (~15us/head) with the PE just under it.
  * pv PSUM is drained by a fast DVE copy; reciprocal/broadcast/normalize
    run on SBUF copies off the critical path.
"""

import sys

if "/opt/trn_rl_repo" not in sys.path:
    sys.path.insert(0, "/opt/trn_rl_repo")

import ml_dtypes
import numpy as np

import concourse.bass as bass  # noqa: F401
import concourse.mybir as mybir
import concourse.tile as tile
from concourse import bacc, bass_utils

F32 = mybir.dt.float32
BF16 = mybir.dt.bfloat16
FP8 = mybir.dt.float8e4
AF = mybir.ActivationFunctionType

P = 128
N_CORES = 8
HEADS = 8
DH = 128
TOK = 4096             # b*n flattened token axis (attention mixes batches!)
SLICE = TOK // N_CORES  # 512 tokens per core
QD = 1024
CD = 768
INNER = 1024
KC = QD // P           # 8 qd chunks
CC = CD // P           # 6 cd chunks
JT = TOK // P          # 32 j-tiles per head
GRP = 3                # j-tiles per exp group ([128, 1536] psum, 3 banks)
NP = HEADS // 2        # head pairs (one AllGather each)
TAU_SCALE = 1.5 * (DH ** -0.5)

_CACHE = {}


def _build():
    nc = bacc.Bacc(num_devices=N_CORES)

    xTs = nc.declare_dram_parameter("xTs", [QD, SLICE], BF16, isOutput=False)
    cTs = nc.declare_dram_parameter("cTs", [CD, SLICE], BF16, isOutput=False)
    Wq = nc.declare_dram_parameter("Wq", [QD, INNER], BF16, isOutput=False)
    Wk = nc.declare_dram_parameter("Wk", [CD, INNER], BF16, isOutput=False)
    Wv = nc.declare_dram_parameter("Wv", [CD, INNER], BF16, isOutput=False)
    Wout = nc.declare_dram_parameter("Wout", [INNER, QD], BF16, isOutput=False)
    boutT = nc.declare_dram_parameter("boutT", [P, KC], F32, isOutput=False)
    yT = nc.declare_dram_parameter("yT", [KC, P, SLICE], F32, isOutput=True)

    rg = [list(range(N_CORES))]

    with tile.TileContext(nc) as tc:
        with (
            tc.tile_pool(name="const", bufs=1) as const,
            tc.tile_pool(name="sb", bufs=1) as sb,
            tc.tile_pool(name="ps", bufs=1, space="PSUM") as ps,
            tc.tile_pool(name="dram", bufs=1, space="DRAM") as dram,
        ):
            kv_in = [dram.tile([2, 2, P, SLICE], BF16, name=f"kv_in{p}")
                     for p in range(NP)]
            kv_g = [dram.tile([N_CORES, 2, 2, P, SLICE], BF16,
                              addr_space="Shared", name=f"kv_g{p}")
                    for p in range(NP)]

            # ---- dummy collective: absorbs the first-collective barrier
            # while the projection phase runs (contents never used)
            dmy_in = dram.tile([P, 8], BF16, name="dmy_in")
            dmy_out = dram.tile([N_CORES, P, 8], BF16, addr_space="Shared",
                                name="dmy_out")
            nc.gpsimd.collective_compute(
                "AllGather", mybir.AluOpType.bypass, replica_groups=rg,
                ins=[dmy_in.opt()], outs=[dmy_out.opt()])

            ones_b = const.tile([P, 1], BF16, name="ones_b")
            nc.vector.memset(ones_b[:], 1.0)
            ones2 = const.tile([P, 4], BF16, name="ones2")
            nc.vector.memset(ones2[:], 1.0)
            nc.vector.memset(ones2[:, 1:3], 0.0)
            bout_sb = const.tile([P, KC], F32, name="bout_sb")
            nc.sync.dma_start(bout_sb[:], boutT[:, :])

            # ---- K/V projection inputs ----
            cts = []
            for k in range(CC):
                t = sb.tile([P, SLICE], BF16, name=f"cts{k}", tag="cts", bufs=CC)
                nc.sync.dma_start(t[:], cTs[k * P:(k + 1) * P, :])
                cts.append(t)
            wkt = []
            for k in range(CC):
                t = sb.tile([P, INNER], BF16, name=f"wkt{k}", tag="wkt", bufs=CC)
                nc.sync.dma_start(t[:], Wk[k * P:(k + 1) * P, :])
                wkt.append(t)
            wvt = []
            for k in range(CC):
                t = sb.tile([P, INNER], BF16, name=f"wvt{k}", tag="wvt", bufs=CC)
                nc.sync.dma_start(t[:], Wv[k * P:(k + 1) * P, :])
                wvt.append(t)

            # ---- per head-pair: K proj, V proj, then AllGather ----
            for p_ in range(NP):
                for hh in range(2):
                    h = 2 * p_ + hh
                    kps = ps.tile([P, GRP * SLICE], F32, name=f"kps{h}",
                                  tag="sim", bufs=2)
                    for k in range(CC):
                        nc.tensor.matmul(kps[:, :SLICE],
                                         wkt[k][:, h * DH:(h + 1) * DH],
                                         cts[k][:],
                                         start=(k == 0), stop=(k == CC - 1))
                    ksb = sb.tile([P, SLICE], BF16, name=f"ksb{h}", tag="ksb",
                                  bufs=4)
                    nc.vector.tensor_copy(ksb[:], kps[:, :SLICE])
                    nc.sync.dma_start(kv_in[p_][0, hh], ksb[:])
                # V: out layout [tok, (tt, hh, dh)] for this pair
                vsb = sb.tile([P, 4 * 2 * DH], BF16, name=f"vsb{p_}",
                              tag="vsb", bufs=2)
                for tt in range(SLICE // P):
                    vps = ps.tile([P, GRP * SLICE], F32, name=f"vps{p_}_{tt}",
                                  tag="sim", bufs=2)
                    for k in range(CC):
                        nc.tensor.matmul(
                            vps[:, :2 * DH],
                            cts[k][:, tt * P:(tt + 1) * P],
                            wvt[k][:, p_ * 2 * DH:(p_ + 1) * 2 * DH],
                            start=(k == 0), stop=(k == CC - 1))
                    nc.vector.tensor_copy(
                        vsb[:, tt * 2 * DH:(tt + 1) * 2 * DH], vps[:, :2 * DH])
                vsb3 = vsb[:].rearrange("p (t c) -> p t c", t=4)
                for hh in range(2):
                    nc.sync.dma_start(kv_in[p_][1, hh],
                                      vsb3[:, :, hh * DH:(hh + 1) * DH])
                nc.gpsimd.collective_compute(
                    "AllGather", mybir.AluOpType.bypass, replica_groups=rg,
                    ins=[kv_in[p_].opt()], outs=[kv_g[p_].opt()])

            # ---- Q projection (all heads); overlaps the AllGathers ----
            xts = []
            for k in range(KC):
                t = sb.tile([P, SLICE], BF16, name=f"xts{k}", tag="xts", bufs=KC)
                nc.sync.dma_start(t[:], xTs[k * P:(k + 1) * P, :])
                xts.append(t)
            wqt = []
            for k in range(KC):
                t = sb.tile([P, INNER], BF16, name=f"wqt{k}", tag="wqt", bufs=KC)
                nc.sync.dma_start(t[:], Wq[k * P:(k + 1) * P, :])
                wqt.append(t)
            # Wout chunks early too (nothing depends on them until the tail,
            # and issuing them now keeps the sync queue free later)
            wo = []
            for cc in range(KC):
                t = sb.tile([P, KC, DH], BF16, name=f"wo{cc}", tag="wo", bufs=KC)
                nc.sync.dma_start(
                    t[:],
                    Wout.ap()[:, cc * DH:(cc + 1) * DH].rearrange(
                        "(k p) c -> p k c", p=P))
                wo.append(t)
            qsb = []
            for h in range(HEADS):
                qps = ps.tile([P, GRP * SLICE], F32, name=f"qps{h}", tag="sim",
                              bufs=2)
                for k in range(KC):
                    nc.tensor.matmul(qps[:, :SLICE],
                                     wqt[k][:, h * DH:(h + 1) * DH],
                                     xts[k][:],
                                     start=(k == 0), stop=(k == KC - 1))
                qt = sb.tile([P, SLICE], BF16, name=f"qsb{h}", tag="qsb",
                             bufs=HEADS)
                nc.vector.tensor_copy(qt[:], qps[:, :SLICE])
                qsb.append(qt)

            # ---- attention, one head at a time over the full 4096 ctx ----
            groups = []
            j0 = 0
            while j0 < JT:
                groups.append(list(range(j0, min(j0 + GRP, JT))))
                j0 += GRP

            osb = [None] * HEADS
            pvs_pair = [None, None]
            rs2_pair = None
            for h in range(HEADS):
                p_, hh = h // 2, h % 2
                kh = sb.tile([P, TOK], BF16, name=f"kh{h}", tag="kh", bufs=3)
                nc.sync.dma_start(
                    kh[:].rearrange("p (r s) -> p r s", r=N_CORES),
                    kv_g[p_][:, 0, hh].rearrange("r p s -> p r s"))
                vh = sb.tile([P, TOK], BF16, name=f"vh{h}", tag="vh", bufs=3)
                nc.sync.dma_start(
                    vh[:].rearrange("p (r s) -> p r s", r=N_CORES),
                    kv_g[p_][:, 1, hh].rearrange("r p s -> p r s"))
                pv_ps = ps.tile([P, SLICE], F32, name=f"pv{h}", tag="pv", bufs=1)
                # bf16 row-sum accumulator, two 512-wide halves accumulated
                # with 1024-wide DVE adds (2x mode, amortized op overhead)
                acc2 = sb.tile([P, 2 * SLICE], BF16, name=f"acc{h}",
                               tag="acc", bufs=2)
                if hh == 0:
                    rs2_ps = ps.tile([2, SLICE], F32, name=f"rs{p_}", tag="rs",
                                     bufs=1)
                    rs2_pair = rs2_ps
                else:
                    rs2_ps = rs2_pair
                for g, js in enumerate(groups):
                    sim_ps = ps.tile([P, GRP * SLICE], F32, name=f"sim{h}_{g}",
                                     tag="sim", bufs=2)
                    for jj, j in enumerate(js):
                        nc.tensor.matmul(
                            sim_ps[:, jj * SLICE:(jj + 1) * SLICE],
                            kh[:, j * P:(j + 1) * P], qsb[h][:],
                            start=True, stop=True)
                    at = sb.tile([P, GRP * SLICE], BF16, name=f"at{h}_{g}",
                                 tag="at", bufs=4)
                    nc.scalar.activation(at[:, :len(js) * SLICE],
                                         sim_ps[:, :len(js) * SLICE], AF.Exp,
                                         scale=TAU_SCALE)
                    for jj, j in enumerate(js):
                        nc.tensor.matmul(pv_ps[:], vh[:, j * P:(j + 1) * P],
                                         at[:, jj * SLICE:(jj + 1) * SLICE],
                                         start=(j == 0), stop=(j == JT - 1))
                    # DVE row-sum accumulation (replaces per-tile ones-matmul)
                    w = min(len(js), 2) * SLICE
                    if g == 0:
                        nc.vector.tensor_copy(acc2[:, :w], at[:, :w])
                    else:
                        nc.vector.tensor_tensor(acc2[:, :w], acc2[:, :w],
                                                at[:, :w],
                                                mybir.AluOpType.add)
                    if len(js) == 3:
                        nc.vector.tensor_tensor(
                            acc2[:, :SLICE], acc2[:, :SLICE],
                            at[:, 2 * SLICE:3 * SLICE], mybir.AluOpType.add)
                # partition-reduce on the PE into the pair's [2,512] rowsum
                st = ones2[:, 2 * hh:2 * hh + 2]
                nc.tensor.matmul(rs2_ps[:], st, acc2[:, :SLICE],
                                 start=(hh == 0), stop=False)
                nc.tensor.matmul(rs2_ps[:], st, acc2[:, SLICE:2 * SLICE],
                                 start=False, stop=(hh == 1))
                # drain pv quickly; normalize per PAIR off the critical path
                pvs = sb.tile([P, SLICE], F32, name=f"pvs{h}", tag="pvs",
                              bufs=2)
                nc.vector.tensor_copy(pvs[:], pv_ps[:])
                pvs_pair[hh] = pvs
                if hh == 1:
                    recip2 = sb.tile([2, SLICE], F32, name=f"recip{p_}",
                                     tag="recip", bufs=2)
                    nc.vector.reciprocal(recip2[:], rs2_ps[:])
                    recip_b = sb.tile([1, SLICE], F32, name=f"recipb{p_}",
                                      tag="recipb", bufs=2)
                    nc.sync.dma_start(recip_b[:], recip2[1:2])
                    for e in range(2):
                        bc = sb.tile([P, SLICE], F32, name=f"bc{h}_{e}",
                                     tag="bc", bufs=2)
                        nc.gpsimd.partition_broadcast(
                            bc[:], recip2[0:1] if e == 0 else recip_b[:])
                        ot = sb.tile([P, SLICE], BF16, name=f"osb{h}_{e}",
                                     tag="osb", bufs=HEADS)
                        nc.vector.tensor_tensor(ot[:], pvs_pair[e][:], bc[:],
                                                mybir.AluOpType.mult)
                        osb[2 * p_ + e] = ot

            # ---- final projection: yT[cc] = Wout[:, cc]^T @ out^T + bout ----
            for cc in range(KC):
                yps = ps.tile([P, SLICE], F32, name=f"yps{cc}",
                              tag=("pv" if cc % 2 == 0 else "rs"), bufs=1)
                for ic in range(HEADS):
                    nc.tensor.matmul(yps[:], wo[cc][:, ic], osb[ic][:],
                                     start=(ic == 0), stop=(ic == HEADS - 1))
                yt = sb.tile([P, SLICE], F32, name=f"yt{cc}", tag="yt", bufs=2)
                nc.scalar.activation(yt[:], yps[:], AF.Identity,
                                     bias=bout_sb[:, cc:cc + 1], scale=1.0)
                nc.sync.dma_start(yT.ap()[cc], yt[:])

    nc.compile()
    return nc


def _get_nc():
    if "nc" not in _CACHE:
        _CACHE["nc"] = _build()
    return _CACHE["nc"]


def _bf16(a):
    return np.ascontiguousarray(np.asarray(a, np.float32).astype(ml_dtypes.bfloat16))


def _prep_in_maps(x, context, Wq, Wk, Wv, Wout, bout):
    x_f = np.asarray(x, dtype=np.float32).reshape(TOK, QD)
    c_f = np.asarray(context, dtype=np.float32).reshape(TOK, CD)
    Wq = _bf16(Wq)
    Wk = _bf16(Wk)
    Wv = _bf16(Wv)
    Wout = _bf16(Wout)
    boutT = np.ascontiguousarray(
        np.asarray(bout, dtype=np.float32).reshape(KC, P).T)
    in_maps = []
    for c in range(N_CORES):
        sl = slice(c * SLICE, (c + 1) * SLICE)
        in_maps.append({
            "xTs": _bf16(x_f[sl].T),
            "cTs": _bf16(c_f[sl].T),
            "Wq": Wq, "Wk": Wk, "Wv": Wv, "Wout": Wout, "boutT": boutT,
        })
    return in_maps


def _assemble(results):
    y = np.empty((TOK, QD), dtype=np.float32)
    for c in range(N_CORES):
        yt = results[c]["yT"]   # [KC, P, SLICE]
        y[c * SLICE:(c + 1) * SLICE] = (
            yt.transpose(2, 0, 1).reshape(SLICE, QD))
    return y.reshape(2, TOK // 2, QD)


def run(inputs, trace=False, **kw):
    nc = _get_nc()
    in_maps = _prep_in_maps(**inputs)
    res = bass_utils.run_bass_kernel_spmd(
        nc, in_maps, core_ids=list(range(N_CORES)), trace=trace, **kw)
    return _assemble(res.results), res


def kernel(**inputs):
    out, _ = run(inputs, trace=False)
    return out
